# revision 44
# baseline (speedup 1.0000x reference)
"""BinaryTreeCell (binary tree LSTM cell) TRN2 Bass kernel.

Full-input contract: kernel(**inputs) takes the unsharded numpy inputs of
reference.setup_inputs() and returns (c, h), each [131072, 256] float32.

Strategy
--------
Data-parallel over the node dimension N=131072 across 8 NeuronCores
(16384 nodes/core); all weights replicated.

All 14 GEMMs (+ the reused W_fx) collapse into ONE matmul per node block:
    z   = [x, lh, rh]                 [N, 768]
    A_g = [W_g.T; Ul_g.T; Ur_g.T]     [768, 256]   per gate g in (u,i,lf,rf,o)
    pre = z @ A + b                   [N, 1280]
The per-gate 256 columns are split into two 128-column halves and packed
as 10 chunks ordered [half0: u,i,lf,rf,o | half1: u,i,lf,rf,o] so the
elementwise stage for one feature half can start as soon as its 5 gates
are done.

On-chip layout is transposed (features on partitions, nodes on the free
dim): the host feeds zT [768, 16384], lcT/rcT [256, 16384] per core and
receives cT/hT [256, 16384], so the kernel needs zero on-chip transposes
and every DMA is wide and contiguous per partition.  Matmuls run in
float32r (full-rate on the PE at free-dim 512, ~1e-4 relative error),
accumulation in fp32 PSUM over K=768 (6 chunks of 128).  Gate
activations run on ScalarE straight out of PSUM with the per-partition
bias folded in; c and h are computed on VectorE (6 tensor_tensor ops per
feature half, plus 2 pre-adds that re-inject the once-computed shared
W_fx projection into the lf/rf gates).
"""

import numpy as np

N_TOTAL = 131072
D = 256
CORES = 8
NP_ = N_TOTAL // CORES          # 16384 nodes per core
KD = 3 * D                      # 768 contraction
KC = KD // 128                  # 6 contraction chunks
GD = 5 * D                      # 1280 gate columns
BM = 512                        # node-block (matmul free dim / PSUM bank)
NBLK = NP_ // BM                # 32 blocks per core

_CACHE = {}


def _build_nc():
    """Build + compile the per-core Bass program (same NEFF for all cores)."""
    import concourse.bass as bass
    import concourse.tile as tile
    from concourse import bacc, mybir

    f32 = mybir.dt.float32
    f32r = mybir.dt.float32r
    AF = mybir.ActivationFunctionType

    nc = bacc.Bacc("TRN2", target_bir_lowering=False, debug=False)

    zT = nc.dram_tensor("zT", [KD, NP_], f32r, kind="ExternalInput").ap()
    lcT = nc.dram_tensor("lcT", [D, NP_], f32, kind="ExternalInput").ap()
    rcT = nc.dram_tensor("rcT", [D, NP_], f32, kind="ExternalInput").ap()
    A = nc.dram_tensor("A", [10, KD, 128], f32r, kind="ExternalInput").ap()
    bias = nc.dram_tensor("bias", [128, 10], f32, kind="ExternalInput").ap()
    cT = nc.dram_tensor("cT", [D, NP_], f32, kind="ExternalOutput").ap()
    hT = nc.dram_tensor("hT", [D, NP_], f32, kind="ExternalOutput").ap()

    with tile.TileContext(nc) as tc:
        with (
            tc.tile_pool(name="wpool", bufs=1) as wpool,
            tc.tile_pool(name="zpool", bufs=4) as zpool,
            tc.tile_pool(name="cpool", bufs=3) as cpool,
            tc.tile_pool(name="gates", bufs=2) as gates,
            tc.tile_pool(name="tmp", bufs=2) as tmp,
            tc.tile_pool(name="outp", bufs=3) as outp,
            tc.tile_pool(name="psum", bufs=6, space="PSUM") as psum,
        ):
            warm = wpool.tile([128, 1], f32, tag="warm")
            nc.gpsimd.memset(warm[:], 0.0)
            warm_o = wpool.tile([128, 1], f32, tag="warm_o")
            nc.scalar.activation(warm_o[:], warm[:], AF.Tanh)
            nc.scalar.activation(warm_o[:], warm[:], AF.Sigmoid)
            b_sb = wpool.tile([128, 10], f32, tag="b")
            nc.gpsimd.dma_start(out=b_sb[:], in_=bias[:])
            A_sb = []
            for n in range(10):
                a_t = wpool.tile([128, KC, 128], f32r, tag=f"A{n}")
                if n < 5:
                    asrc = A[n].rearrange("(kc p) m -> p kc m", p=128)
                    if n == 3:
                        nc.scalar.dma_start(out=a_t[:, 2:, :], in_=asrc[:, 2:, :])
                    else:
                        nc.scalar.dma_start(out=a_t[:, 0:3, :], in_=asrc[:, 0:3, :])
                        nc.scalar.dma_start(out=a_t[:, 3:, :], in_=asrc[:, 3:, :])
                A_sb.append(a_t)
            deferred_a = [False]

            def load_rest_of_A():
                if deferred_a[0]:
                    return
                deferred_a[0] = True
                for n in range(5, 10):
                    asrc = A[n].rearrange("(kc p) m -> p kc m", p=128)
                    if n == 8:
                        nc.scalar.dma_start(out=A_sb[n][:, 2:, :], in_=asrc[:, 2:, :])
                    else:
                        nc.scalar.dma_start(
                            out=A_sb[n][:, 0:3, :], in_=asrc[:, 0:3, :]
                        )
                        nc.scalar.dma_start(out=A_sb[n][:, 3:, :], in_=asrc[:, 3:, :])

            blocks = [(i * BM, BM) for i in range(NBLK - 1)]
            last = (NBLK - 1) * BM
            blocks += [(last, BM // 2), (last + BM // 2, BM // 2)]
            for blk, (m0, bm) in enumerate(blocks):
                z_sb = zpool.tile([128, KC, bm], f32r, tag="z")
                zsrc = zT[:, m0:m0 + bm].rearrange("(kc p) m -> p kc m", p=128)
                nc.sync.dma_start(out=z_sb[:, 0:3, :], in_=zsrc[:, 0:3, :])
                nc.sync.dma_start(out=z_sb[:, 3:, :], in_=zsrc[:, 3:, :])
                lc_sb = cpool.tile([128, 2, bm], f32, tag="lc")
                nc.gpsimd.dma_start(
                    out=lc_sb[:],
                    in_=lcT[:, m0:m0 + bm].rearrange("(f p) m -> p f m", p=128),
                )
                rc_sb = cpool.tile([128, 2, bm], f32, tag="rc")
                nc.gpsimd.dma_start(
                    out=rc_sb[:],
                    in_=rcT[:, m0:m0 + bm].rearrange("(f p) m -> p f m", p=128),
                )

                for f in range(2):
                    g_sb = []
                    # u, i: full K=768 accumulation
                    for g in (0, 1):
                        n = 5 * f + g
                        ps = psum.tile([128, bm], f32, tag="mm")
                        for k in range(KC):
                            nc.tensor.matmul(
                                ps[:], A_sb[n][:, k, :], z_sb[:, k, :],
                                start=(k == 0), stop=(k == KC - 1),
                            )
                        gt = gates.tile([128, bm], f32, tag=f"g{g}")
                        nc.scalar.activation(
                            gt[:], ps[:],
                            AF.Tanh if g == 0 else AF.Sigmoid,
                            bias=b_sb[:, n:n + 1],
                        )
                        g_sb.append(gt)
                        load_rest_of_A()
                    n_lf = 5 * f + 2
                    n_rf = 5 * f + 3
                    if blk == len(blocks) - 1:
                        # final block: full K=768 for lf/rf — no DVE pre-add
                        # in the kernel's exposed tail chain (rf x-chunks are
                        # W_fx duplicates, valid contraction over all 768)
                        for g, n in ((2, n_lf), (3, n_rf)):
                            ps = psum.tile([128, bm], f32, tag="mm")
                            for k in range(KC):
                                nc.tensor.matmul(
                                    ps[:], A_sb[n_lf if k < 2 else n][:, k, :],
                                    z_sb[:, k, :],
                                    start=(k == 0), stop=(k == KC - 1),
                                )
                            gt = gates.tile([128, bm], f32, tag=f"g{g}")
                            nc.scalar.activation(
                                gt[:], ps[:], AF.Sigmoid, bias=b_sb[:, n_lf:n_lf + 1],
                            )
                            g_sb.append(gt)
                    else:
                        # fx computed once (x chunks of the lf column block)
                        ps_fx = psum.tile([128, bm], f32, tag="mm")
                        for k in (0, 1):
                            nc.tensor.matmul(
                                ps_fx[:], A_sb[n_lf][:, k, :], z_sb[:, k, :],
                                start=(k == 0), stop=(k == 1),
                            )
                        fx_sb = gates.tile([128, bm], f32, tag="fx")
                        nc.scalar.activation(
                            fx_sb[:], ps_fx[:], AF.Identity,
                            bias=b_sb[:, n_lf:n_lf + 1],
                        )
                        # lf, rf: only the lh/rh chunks, then + fx on DVE
                        for g, n in ((2, n_lf), (3, n_rf)):
                            ps = psum.tile([128, bm], f32, tag="mm")
                            for k in (2, 3, 4, 5):
                                nc.tensor.matmul(
                                    ps[:], A_sb[n][:, k, :], z_sb[:, k, :],
                                    start=(k == 2), stop=(k == 5),
                                )
                            pre = tmp.tile([128, bm], f32, tag=f"pre{g}")
                            nc.vector.tensor_add(pre[:], ps[:], fx_sb[:])
                            gt = gates.tile([128, bm], f32, tag=f"g{g}")
                            nc.scalar.activation(gt[:], pre[:], AF.Sigmoid)
                            g_sb.append(gt)
                    # o: full K=768
                    n = 5 * f + 4
                    ps = psum.tile([128, bm], f32, tag="mm")
                    for k in range(KC):
                        nc.tensor.matmul(
                            ps[:], A_sb[n][:, k, :], z_sb[:, k, :],
                            start=(k == 0), stop=(k == KC - 1),
                        )
                    gt = gates.tile([128, bm], f32, tag="g4")
                    nc.scalar.activation(
                        gt[:], ps[:], AF.Sigmoid, bias=b_sb[:, n:n + 1],
                    )
                    g_sb.append(gt)

                    u_t, i_t, lf_t, rf_t, o_t = g_sb
                    t1 = tmp.tile([128, bm], f32, tag="t1")
                    nc.vector.tensor_mul(t1[:], i_t[:], u_t[:])
                    t2 = tmp.tile([128, bm], f32, tag="t2")
                    nc.vector.tensor_mul(t2[:], lf_t[:], lc_sb[:, f, :])
                    t3 = tmp.tile([128, bm], f32, tag="t3")
                    nc.vector.tensor_mul(t3[:], rf_t[:], rc_sb[:, f, :])
                    nc.vector.tensor_add(t1[:], t1[:], t2[:])
                    c_t = outp.tile([128, bm], f32, tag="c")
                    nc.vector.tensor_add(c_t[:], t1[:], t3[:])
                    eng_out = nc.sync if bm < BM else nc.gpsimd
                    eng_out.dma_start(
                        out=cT[f * 128:(f + 1) * 128, m0:m0 + bm], in_=c_t[:]
                    )
                    tc_t = tmp.tile([128, bm], f32, tag="tc")
                    nc.scalar.activation(tc_t[:], c_t[:], AF.Tanh)
                    h_t = outp.tile([128, bm], f32, tag="h")
                    nc.vector.tensor_mul(h_t[:], o_t[:], tc_t[:])
                    eng_out.dma_start(
                        out=hT[f * 128:(f + 1) * 128, m0:m0 + bm], in_=h_t[:]
                    )

    nc.compile()
    return nc


def _pack_weights(W_cx, b_cx, W_ox, b_ox, W_fx, b_fx, W_ix, b_ix,
                  U_ilh, U_irh, U_lflh, U_lfrh, U_rflh, U_rfrh,
                  U_ulh, U_urh, U_olh, U_orh):
    """A [10, 768, 128]: one [768, 128] column chunk per (half, gate),
    ordered [half0: u,i,lf,rf,o | half1: ...]; bias [128, 10] matches.
    Chunks 3 and 8 (rf) duplicate W_fx.T in rows 0:256 — the kernel never
    reads those rows except on the final block, where it substitutes the
    lf chunk's copy."""
    gates = [
        (W_cx, U_ulh, U_urh, b_cx),   # u
        (W_ix, U_ilh, U_irh, b_ix),   # i
        (W_fx, U_lflh, U_lfrh, b_fx),  # lf
        (W_fx, U_rflh, U_rfrh, b_fx),  # rf
        (W_ox, U_olh, U_orh, b_ox),   # o
    ]
    A = np.empty((10, KD, 128), dtype=np.float32)
    bias = np.empty((128, 10), dtype=np.float32)
    for g, (W, Ul, Ur, b) in enumerate(gates):
        Ag = np.concatenate([W.T, Ul.T, Ur.T], axis=0)  # [768, 256]
        for f in range(2):
            n = 5 * f + g
            A[n] = Ag[:, f * 128:(f + 1) * 128]
            bias[:, n] = b[f * 128:(f + 1) * 128]
    return np.ascontiguousarray(A), np.ascontiguousarray(bias)


def kernel(x, lc, lh, rc, rh,
           W_cx, b_cx, W_ox, b_ox, W_fx, b_fx, W_ix, b_ix,
           U_ilh, U_irh, U_lflh, U_lfrh, U_rflh, U_rfrh,
           U_ulh, U_urh, U_olh, U_orh):
    from concourse.bass_utils import run_bass_kernel_spmd

    x = np.asarray(x, dtype=np.float32)
    lc = np.asarray(lc, dtype=np.float32)
    lh = np.asarray(lh, dtype=np.float32)
    rc = np.asarray(rc, dtype=np.float32)
    rh = np.asarray(rh, dtype=np.float32)

    A, bias = _pack_weights(
        np.asarray(W_cx, np.float32), np.asarray(b_cx, np.float32),
        np.asarray(W_ox, np.float32), np.asarray(b_ox, np.float32),
        np.asarray(W_fx, np.float32), np.asarray(b_fx, np.float32),
        np.asarray(W_ix, np.float32), np.asarray(b_ix, np.float32),
        np.asarray(U_ilh, np.float32), np.asarray(U_irh, np.float32),
        np.asarray(U_lflh, np.float32), np.asarray(U_lfrh, np.float32),
        np.asarray(U_rflh, np.float32), np.asarray(U_rfrh, np.float32),
        np.asarray(U_ulh, np.float32), np.asarray(U_urh, np.float32),
        np.asarray(U_olh, np.float32), np.asarray(U_orh, np.float32),
    )

    if "nc" not in _CACHE:
        _CACHE["nc"] = _build_nc()
    nc = _CACHE["nc"]

    in_maps = []
    for c in range(CORES):
        sl = slice(c * NP_, (c + 1) * NP_)
        zTc = np.empty((KD, NP_), dtype=np.float32)
        zTc[0:D] = x[sl].T
        zTc[D:2 * D] = lh[sl].T
        zTc[2 * D:3 * D] = rh[sl].T
        in_maps.append({
            "zT": zTc,
            "lcT": np.ascontiguousarray(lc[sl].T),
            "rcT": np.ascontiguousarray(rc[sl].T),
            "A": A,
            "bias": bias,
        })

    import time as _time
    t0 = _time.time()
    res = None
    for attempt, backoff_s in ((0, 15), (1, 45), (2, None)):
        try:
            res = run_bass_kernel_spmd(nc, in_maps, core_ids=list(range(CORES)))
            break
        except Exception:
            # transient device wedge (e.g. NRT_EXEC_UNIT_UNRECOVERABLE):
            # back off and retry; re-raise on the final attempt
            if backoff_s is None:
                raise
            _time.sleep(backoff_s)
    t1 = _time.time()
    _CACHE["last_wall_s"] = t1 - t0
    _CACHE["last_exec_ns"] = res.exec_time_ns

    c_out = np.empty((N_TOTAL, D), dtype=np.float32)
    h_out = np.empty((N_TOTAL, D), dtype=np.float32)
    for ci in range(CORES):
        sl = slice(ci * NP_, (ci + 1) * NP_)
        c_out[sl] = res.results[ci]["cT"].T
        h_out[sl] = res.results[ci]["hT"].T
    return c_out, h_out
